# revision 8
# baseline (speedup 1.0000x reference)
"""Fused single-kernel AtomGCNLayer for TRN2 (v2 design).

Key changes vs baseline:
  - Scatter matmuls swapped: stationary = msg [128,16] (LDW 13ns), moving =
    one-hot [128,64] -> feature-major aggT [16,64] written to 32-aligned
    PE column strips (4 windows per PSUM bank partition dim).
  - Main matmul pairs 2 edge-blocks per instruction (98 partitions,
    block-diagonal weights): halves LDWEIGHTS work, FWL stays active
    (128 weight cols), and DMA uses 13/16 engines instead of 7/16.
  - Skip matmul (M=32) is the accumulation starter and zeroes the 16
    garbage partition rows per strip; it reads x directly from the packed
    residual tensor xpk (no separate xsk input).
  - One-hot built once per superchunk in a single DVE op.
  - h never goes to DRAM: stays in SBUF (bf16), BN stats reduced on-chip
    (ACT accum_out) + 8-core AllReduce; scale/shift computed on device;
    y = relu(h*scale+shift) + x emitted bf16.
"""

import math

import numpy as np
import ml_dtypes

from concourse import bacc, mybir
import concourse.tile as tile
from concourse.bass_utils import run_bass_kernel_spmd

BF16 = ml_dtypes.bfloat16

N = 500000
E = 5000000
D = 16
NC = 8
W = 64              # nodes per window
PW = 1024           # windows per core
NODES_C = W * PW    # 65536
NPAD = NC * NODES_C
NWIN = NC * PW
NBANK = 32          # PSUM agg banks per core (32 windows per bank)
WPB = 32            # windows per bank
SC = 8              # groups per superchunk
BN_EPS = 1e-5

_nc_cache = {}


def _build(B, reps=1, ablate=()):
    ablate = set(ablate)
    no_coll = "no_coll" in ablate
    no_y = "no_y" in ablate
    no_scat = "no_scat" in ablate
    no_msg = "no_msg" in ablate
    no_mm = "no_mm" in ablate
    no_oh2 = "no_oh2" in ablate
    dbl_scat = "dbl_scat" in ablate
    if no_mm:
        assert no_msg
    GB = 2 * B                  # blocks per group (2 windows)
    NP = GB // 2                # matmul pairs per group
    ngroups = PW // 2           # 512
    nsc = ngroups // SC         # 64
    SPB = nsc // NBANK          # superchunks per bank (2)
    S_c = PW * B * 128

    bf = mybir.dt.bfloat16
    f32 = mybir.dt.float32
    SIG = mybir.ActivationFunctionType.Sigmoid
    SQT = mybir.ActivationFunctionType.Sqrt
    SQR = mybir.ActivationFunctionType.Square
    CPY = mybir.ActivationFunctionType.Copy
    RLU = mybir.ActivationFunctionType.Relu
    MUL = mybir.AluOpType.mult
    SUB = mybir.AluOpType.subtract
    ADD = mybir.AluOpType.add
    EQ = mybir.AluOpType.is_equal

    nc = bacc.Bacc(None, target_bir_lowering=False, debug=True)
    INP2 = nc.dram_tensor("inp2", [98, S_c // 256, 128], bf, kind="ExternalInput")
    TREL = nc.dram_tensor("trel", [128, nsc, SC, GB], bf, kind="ExternalInput")
    IOTA = nc.dram_tensor("iota", [128, SC, GB, W], bf, kind="ExternalInput")
    XPK = nc.dram_tensor("xpk", [128, NBANK, 8, 64], bf, kind="ExternalInput")
    WT2 = nc.dram_tensor("wt2", [98, 64], bf, kind="ExternalInput")
    WSB4 = nc.dram_tensor("wsb4", [128, 32], bf, kind="ExternalInput")
    SEL1 = nc.dram_tensor("sel1", [128, 16], f32, kind="ExternalInput")
    REP16 = nc.dram_tensor("rep16", [16, 128], f32, kind="ExternalInput")
    GB4 = nc.dram_tensor("gb4", [128, 2], f32, kind="ExternalInput")
    Y = nc.dram_tensor("y", [NBANK, 128, 8, 64], bf, kind="ExternalOutput")

    with tile.TileContext(nc) as tc:
        with (
            tc.tile_pool(name="const", bufs=1) as cpool,
            tc.tile_pool(name="sbuf", bufs=3) as pool,
            tc.tile_pool(name="icp", bufs=2) as ipool,
            tc.tile_pool(name="ohp", bufs=4) as opool,
            tc.tile_pool(name="trp", bufs=1) as tpool,
            tc.tile_pool(name="msgp", bufs=3) as mpool,
            tc.tile_pool(name="hs", bufs=1) as hpool,
            tc.tile_pool(name="st", bufs=1) as spool,
            tc.tile_pool(name="pm", bufs=4, space="PSUM") as pm,
            tc.tile_pool(name="pa", bufs=2, space="PSUM") as pa,
            tc.tile_pool(name="pst", bufs=1, space="PSUM") as pst,
            tc.tile_pool(name="dram", bufs=2, space="DRAM") as dram,
        ):
            wt2 = cpool.tile([98, 64], bf)
            nc.sync.dma_start(wt2[:], WT2[:])
            wsb4 = cpool.tile([128, 32], bf)
            nc.sync.dma_start(wsb4[:], WSB4[:])
            it8 = cpool.tile([128, SC, GB, W], bf)
            nc.sync.dma_start(it8[:], IOTA[:])
            trel = cpool.tile([128, nsc, SC, GB], bf)
            nc.sync.dma_start(trel[:], TREL[:])
            xpk = cpool.tile([128, NBANK, 8, 64], bf)
            nc.sync.dma_start(xpk[:], XPK[:])
            sel1 = cpool.tile([128, 16], f32)
            nc.scalar.dma_start(sel1[:], SEL1[:])
            rep16 = cpool.tile([16, 128], f32)
            nc.scalar.dma_start(rep16[:], REP16[:])
            gb4 = cpool.tile([128, 2], f32)
            nc.scalar.dma_start(gb4[:], GB4[:])

            Hs = hpool.tile([128, NBANK, 8, 64], bf)
            msgC = None
            ohC = None
            if no_msg:
                msgC = cpool.tile([128, SPB * SC, NP, 32], bf)
                nc.gpsimd.memset(msgC[:], 0.0)
            if no_msg or no_oh2:
                ohC = cpool.tile([128, SC, GB, W], bf)
                nc.gpsimd.memset(ohC[:], 0.0)

            for rep in range(reps):
                sts = spool.tile([128, NBANK], f32, tag="sts")
                stq = spool.tile([128, NBANK], f32, tag="stq")
                def emit_bank_b(k2, agg, msg_bk, ohs):
                    # merged skip matmuls: one per 32-strip, N=512
                    for s4 in range(4):
                        p0 = 32 * s4
                        nc.tensor.matmul(
                            agg[p0:p0 + 32, :, :],
                            lhsT=wsb4[p0:p0 + 17, :],
                            rhs=xpk[p0:p0 + 17, k2, :, :],
                            start=True, stop=False,
                            tile_position=(p0, p0),
                            skip_group_check=True)
                    if not no_scat:
                        for gg in range(SPB * SC):
                            ss, g = gg // SC, gg % SC
                            oh_t = ohs[ss]
                            for wi in range(2):
                                w_loc = gg * 2 + wi
                                s4 = w_loc % 4
                                cb = w_loc // 4
                                p0 = 32 * s4
                                for rpt in range(2 if dbl_scat else 1):
                                  for b6 in range(B):
                                    blk = wi * B + b6
                                    bp, par = blk // 2, blk % 2
                                    nc.tensor.matmul(
                                        agg[p0:p0 + 16, cb, :],
                                        lhsT=msg_bk[:, gg, bp,
                                                    16 * par:16 * par + 16],
                                        rhs=oh_t[:, g, blk, :],
                                        start=False,
                                        stop=(b6 == B - 1 and
                                              rpt == (1 if dbl_scat else 0)),
                                        tile_position=(0, p0),
                                        skip_group_check=True)
                    nc.scalar.activation(Hs[:, k2], agg[:], func=CPY,
                                         accum_out=sts[:, k2:k2 + 1])
                    hsq = pool.tile([128, 8, 64], f32, tag="hsq")
                    nc.scalar.activation(hsq[:], agg[:], func=SQR,
                                         accum_out=stq[:, k2:k2 + 1])

                prev_bank = None
                for k2 in range(NBANK):
                    agg = pa.tile([128, 8, 64], f32, space="PSUM", tag="agg")
                    if no_msg:
                        msg_bk = msgC
                    else:
                        msg_bk = mpool.tile([128, SPB * SC, NP, 32], bf,
                                            tag="msg", name="msg_bk")
                    ohs = []
                    for ss in range(SPB):
                        sc = k2 * SPB + ss
                        ic2 = ipool.tile([98, SC * NP, 128], bf, tag="ic")
                        eng = nc.sync if sc % 2 == 0 else nc.scalar
                        eng.dma_start(ic2[:], INP2[:, sc * SC * NP:(sc + 1) * SC * NP, :])
                        if no_msg or no_oh2:
                            ohs.append(ohC)
                        else:
                            oh = opool.tile([128, SC, GB, W], bf, tag="oh")
                            nc.vector.tensor_tensor(
                                oh[:],
                                trel[:, sc].unsqueeze(3)
                                    .to_broadcast([128, SC, GB, W]),
                                it8[:],
                                op=EQ,
                            )
                            ohs.append(oh)
                        for g in range(SC):
                            if no_mm:
                                continue
                            mm2 = pm.tile([128, NP, 64], f32, space="PSUM",
                                          tag="mm")
                            for bp in range(NP):
                                nc.tensor.matmul(mm2[:, bp, :],
                                                 lhsT=ic2[:, g * NP + bp, :],
                                                 rhs=wt2[:], start=True, stop=True)
                            if not no_msg:
                                sg = pool.tile([128, NP, 32], bf, tag="sg")
                                nc.scalar.activation(sg[:], mm2[:, :, 0:32], func=SIG)
                                nc.vector.tensor_tensor(
                                    msg_bk[:, ss * SC + g], sg[:],
                                    mm2[:, :, 32:64], op=MUL)
                    if prev_bank is not None:
                        emit_bank_b(*prev_bank)
                    prev_bank = (k2, agg, msg_bk, ohs)
                emit_bank_b(*prev_bank)

                # --- BN stats: local reduce + cross-core allreduce ---
                stot = pool.tile([128, 2], f32, tag="stot")
                nc.vector.tensor_reduce(stot[:, 0:1], sts[:],
                                        axis=mybir.AxisListType.X, op=ADD)
                nc.vector.tensor_reduce(stot[:, 1:2], stq[:],
                                        axis=mybir.AxisListType.X, op=ADD)
                bin_ = dram.tile([128, 2], f32, tag="bin")
                bout = dram.tile([128, 2], f32, tag="bout")
                nc.gpsimd.dma_start(bin_[:], stot[:])
                if no_coll:
                    nc.gpsimd.dma_start(bout[:], bin_[:])
                else:
                    nc.gpsimd.collective_compute(
                        "AllReduce", mybir.AluOpType.add,
                        replica_groups=[list(range(NC))],
                        ins=[bin_.opt()], outs=[bout.opt()],
                    )
                sall = pool.tile([128, 2], f32, tag="sall")
                nc.gpsimd.dma_start(sall[:], bout[:])

                # --- fold strips: per-feature totals, broadcast back ---
                st16p = pst.tile([16, 2], f32, space="PSUM", tag="st16p")
                nc.tensor.matmul(st16p[:], lhsT=sel1[:], rhs=sall[:],
                                 start=True, stop=True)
                st16 = pool.tile([16, 2], f32, tag="st16")
                nc.vector.tensor_copy(st16[:], st16p[:])
                stb = pst.tile([128, 2], f32, space="PSUM", tag="stb")
                nc.tensor.matmul(stb[:], lhsT=rep16[:], rhs=st16[:],
                                 start=True, stop=True)

                # --- scale/shift: cols 0 mean,1 ex2,2 m2,3 var,4 sd,5 rinv,
                #     6 scl,7 sft ---
                s8 = pool.tile([128, 8], f32, tag="s8")
                nc.vector.tensor_scalar(s8[:, 0:2], stb[:, 0:2], 1.0 / N, None,
                                        op0=MUL)
                nc.vector.tensor_tensor(s8[:, 2:3], s8[:, 0:1], s8[:, 0:1], op=MUL)
                nc.vector.tensor_tensor(s8[:, 3:4], s8[:, 1:2], s8[:, 2:3], op=SUB)
                nc.vector.tensor_scalar(s8[:, 3:4], s8[:, 3:4], BN_EPS, None,
                                        op0=ADD)
                nc.scalar.activation(s8[:, 4:5], s8[:, 3:4], func=SQT)
                nc.vector.reciprocal(s8[:, 5:6], s8[:, 4:5])
                nc.vector.tensor_tensor(s8[:, 6:7], s8[:, 5:6], gb4[:, 0:1], op=MUL)
                nc.vector.tensor_tensor(s8[:, 7:8], s8[:, 0:1], s8[:, 6:7], op=MUL)
                nc.vector.tensor_tensor(s8[:, 7:8], gb4[:, 1:2], s8[:, 7:8], op=SUB)

                # --- y = relu(h*scl + sft) + x ---
                for k2 in range(0 if not no_y else NBANK, NBANK):
                    yr = pool.tile([128, 8, 64], bf, tag="yr")
                    nc.scalar.activation(yr[:], Hs[:, k2], func=RLU,
                                         scale=s8[:, 6:7], bias=s8[:, 7:8])
                    yo = pool.tile([128, 8, 64], bf, tag="yo")
                    nc.vector.tensor_tensor(yo[:], yr[:], xpk[:, k2], op=ADD)
                    nc.gpsimd.dma_start(Y[k2], yo[:])
    nc.compile()
    return nc


def host_prep(x, edge_index, edge_attr):
    src = np.asarray(edge_index[0], dtype=np.int64)
    tgt = np.asarray(edge_index[1], dtype=np.int64)
    x = np.asarray(x, dtype=np.float32)
    ea = np.asarray(edge_attr, dtype=np.float32)

    perm = np.argsort(tgt, kind="stable")
    tgt_s = tgt[perm]
    src_s = src[perm]
    wid = tgt_s // W
    counts = np.bincount(wid, minlength=NWIN)
    B = max(1, int(math.ceil(counts.max() / 128)))
    if B % 2:
        B += 1
    assert 2 * B * 32 <= 512, f"B={B} too large for paired groups"
    S_w = 128 * B
    S = NWIN * S_w
    S_c = PW * S_w
    starts = np.zeros(NWIN + 1, np.int64)
    starts[1:] = np.cumsum(counts)
    slots = wid * S_w + (np.arange(E, dtype=np.int64) - starts[wid])

    GB = 2 * B
    ngroups = PW // 2
    nsc = ngroups // SC

    x16 = x.astype(BF16)
    pay = np.zeros((S, 48), BF16)
    pay[slots, 0:16] = x16[tgt_s]
    pay[slots, 16:32] = x16[src_s]
    pay[slots, 32:48] = ea[perm].astype(BF16)

    trel = np.full(S, -1.0, np.float32)
    trel[slots] = (tgt_s % W).astype(np.float32)
    trel16 = trel.astype(BF16)

    xpad = np.zeros((NPAD, D), np.float32)
    xpad[:N] = x
    mask = np.zeros(NPAD, np.float32)
    mask[:N] = 1.0

    iota = np.broadcast_to(
        np.arange(W, dtype=np.float32).astype(BF16).reshape(1, 1, 1, W),
        (128, SC, GB, W)).copy()

    # xpk: [NC, 128, NBANK, 8, 64]; partition 32*s4+q, bank k2, col-block cb,
    # node-in-window v; window w = k2*32 + 4*cb + s4
    xq = xpad.reshape(NC, NBANK, 8, 4, W, D)       # c, k2, cb, s4, v, q
    mq = mask.reshape(NC, NBANK, 8, 4, W)
    xpk = np.zeros((NC, 4, 32, NBANK, 8, W), np.float32)
    xpk[:, :, 0:16] = xq.transpose(0, 3, 5, 1, 2, 4)
    xpk[:, :, 16] = mq.transpose(0, 3, 1, 2, 4)
    xpk16 = xpk.reshape(NC, 128, NBANK, 8, W).astype(BF16)

    in_maps = []
    for c in range(NC):
        blocks = pay[c * S_c:(c + 1) * S_c].reshape(S_c // 128, 128, 48)
        inp2 = np.empty((98, S_c // 256, 128), BF16)
        inp2[0:48] = blocks[0::2].transpose(2, 0, 1)
        inp2[48] = BF16(1.0)
        inp2[49:97] = blocks[1::2].transpose(2, 0, 1)
        inp2[97] = BF16(1.0)
        trel_c = (trel16[c * S_c:(c + 1) * S_c]
                  .reshape(nsc, SC, GB, 128).transpose(3, 0, 1, 2).copy())
        in_maps.append({
            "inp2": inp2, "trel": trel_c, "iota": iota, "xpk": xpk16[c],
        })
    return B, in_maps, xpad


def weight_arrays(Wk, bk, Wq, bq, Wv, bv, We, Ws, bs, bias, gamma, beta):
    wgt = np.zeros((49, 32), np.float32)
    wgt[0:16, 0:16] = Wk
    wgt[16:32, 0:16] = Wq
    wgt[32:48, 0:16] = We
    wgt[48, 0:16] = bk + bq
    wgt[16:32, 16:32] = Wv
    wgt[48, 16:32] = bv
    wt2 = np.zeros((98, 64), np.float32)
    wt2[0:49, 0:16] = wgt[:, 0:16]
    wt2[0:49, 32:48] = wgt[:, 16:32]
    wt2[49:98, 16:32] = wgt[:, 0:16]
    wt2[49:98, 48:64] = wgt[:, 16:32]

    wsb = np.zeros((17, 16), np.float32)
    wsb[0:16] = Ws
    wsb[16] = bs + bias
    wsb4 = np.zeros((128, 32), np.float32)
    for s in range(4):
        wsb4[32 * s:32 * s + 17, 0:16] = wsb

    p = np.arange(128)
    sel1 = (p[:, None] % 32 == np.arange(16)[None, :]).astype(np.float32)
    rep16 = sel1.T.copy()
    gb4 = np.zeros((128, 2), np.float32)
    q = p % 32
    valid = q < 16
    gb4[valid, 0] = gamma[q[valid]]
    gb4[valid, 1] = beta[q[valid]]
    return (wt2.astype(BF16), wsb4.astype(BF16), sel1, rep16, gb4)


def untile_y(y_all):
    # y_all: [NC, NBANK, 128, 8, 64] -> [NPAD, 16]
    yv = y_all.reshape(NC, NBANK, 128, 8, W).astype(np.float32)
    yv = yv.reshape(NC, NBANK, 4, 32, 8, W)[:, :, :, 0:16]   # c,k2,s4,q,cb,v
    return yv.transpose(0, 1, 4, 2, 5, 3).reshape(NPAD, D)


def kernel(**inputs):
    x = np.asarray(inputs["x"], np.float32)
    B, in_maps, xpad = host_prep(x, inputs["edge_index"], inputs["edge_attr"])
    wt2, wsb4, sel1, rep16, gb4 = weight_arrays(
        *[np.asarray(inputs[k], np.float32) for k in
          ["Wk", "bk", "Wq", "bq", "Wv", "bv", "We", "Ws", "bs", "bias",
           "gamma", "beta"]])
    for m in in_maps:
        m["wt2"] = wt2
        m["wsb4"] = wsb4
        m["sel1"] = sel1
        m["rep16"] = rep16
        m["gb4"] = gb4

    if ("v2", B) not in _nc_cache:
        _nc_cache[("v2", B)] = _build(B)
    nc = _nc_cache[("v2", B)]
    res = run_bass_kernel_spmd(nc, in_maps, list(range(NC)))
    y_all = np.stack([res.results[c]["y"] for c in range(NC)])
    return untile_y(y_all)[:N].astype(np.float32)
